# revision 31
# baseline (speedup 1.0000x reference)
"""Trainium2 Bass kernel for nn_ChimeraV2Block (dual-softmax differential
sliding-window attention block, B=1 S=2048 D=2048, 16 q-heads / 4 kv-heads,
head_dim 128, window 512).

Sharding: tensor-parallel over heads across 8 NeuronCores. Core c owns
q-heads {2c, 2c+1} and kv-head c//2 (GQA groups align with the split).
Wq/Wk/Wv column-sharded, Wo row-sharded; the 8 fp32 partial outputs are
summed on the host (the "all-reduce").

v2: fused streaming pipeline (proj -> RoPE -> attention -> out-proj per
512-seq chunk), edge-tile-only band masks, engine rebalance (exp + PSUM
copies on Activation, g0/cneg/dtmp on Pool, RoPE on DVE in bf16), out-proj
lagged one q-tile behind attention, PSUM packed into exactly 8 banks.
"""

import sys

if "/opt/trn_rl_repo" not in sys.path:
    sys.path.insert(0, "/opt/trn_rl_repo")

import numpy as np
import ml_dtypes

BF = ml_dtypes.bfloat16

S = 2048
D = 2048
H = 16
HK = 4
HD = 128
WIN = 512
THETA = 10000.0
N_CORES = 8
NQT = S // 128          # 16 q row-tiles
NKT = D // 128          # 16 contraction tiles for the projections
NCH = 4                 # 512-seq chunks
NEG = -1.0e30

_CACHE = {}


def _tables():
    """RoPE tables [128, S] bf16 with head-dim-duplicated frequencies
    (row p uses invf[p % 64]). The attention scale 1/sqrt(64) is folded
    into Wq host-side, so q and k share these tables."""
    invf = 1.0 / (THETA ** (np.arange(0, HD, 2, dtype=np.float64) / HD))  # [64]
    t = np.arange(S, dtype=np.float64)
    fr = np.outer(invf, t)  # [64, S]
    cosf = np.concatenate([np.cos(fr)] * 2, axis=0)
    # sign folded into the sin table: out = f*cos + swap(f)*sin_signed, where
    # the table is read at the *swapped operand's* partitions: row p in 0:64
    # (+sin) feeds out rows 64:128; row p in 64:128 (-sin) feeds out rows 0:64.
    sinf = np.concatenate([np.sin(fr), -np.sin(fr)], axis=0)
    return (np.ascontiguousarray(cosf, dtype=BF),
            np.ascontiguousarray(sinf, dtype=BF))


def _masks():
    """Edge-tile masks [128,128]: within a window of kw 128-col key tiles
    only the oldest tile (keys c<=p banned: outside the 512 window) and the
    newest tile (keys c>p banned: causal) are partially masked."""
    p = np.arange(128)[:, None]
    c = np.arange(128)[None, :]
    m_li = np.where(c <= p, NEG, 0.0).astype(BF)   # oldest tile (qi >= 4)
    m_c = np.where(c > p, NEG, 0.0).astype(BF)     # newest tile (causal)
    return m_li, m_c


def _build_program():
    import concourse.bacc as bacc
    import concourse.tile as tile
    from concourse import mybir

    bf = mybir.dt.bfloat16
    f32 = mybir.dt.float32
    f16 = mybir.dt.float16
    EXP = mybir.ActivationFunctionType.Exp
    MULT = mybir.AluOpType.mult
    ADD = mybir.AluOpType.add
    MAX = mybir.AluOpType.max

    nc = bacc.Bacc("TRN2", target_bir_lowering=False, debug=False,
                   num_devices=N_CORES)

    xt_d = nc.dram_tensor("xt", [128, NCH, NKT, 512], bf, kind="ExternalInput")
    wq_d = nc.dram_tensor("wq", [128, NKT, 2, 128], bf, kind="ExternalInput")
    wk_d = nc.dram_tensor("wk", [128, NKT, 128], bf, kind="ExternalInput")
    wv_d = nc.dram_tensor("wv", [128, NKT, 128], bf, kind="ExternalInput")
    wo_d = nc.dram_tensor("wo", [128, 2, D], bf, kind="ExternalInput")
    lamn_d = nc.dram_tensor("lamn", [1, 2], f32, kind="ExternalInput")
    out_d = nc.dram_tensor("outp", [S, D], f16, kind="ExternalOutput")

    tcos_np, tsin_np = _tables()
    mli_np, mc_np = _masks()
    tcos_d = nc.inline_tensor(tcos_np, "tab_cos")
    tsin_d = nc.inline_tensor(tsin_np, "tab_sin")
    mli_d = nc.inline_tensor(mli_np, "mask_li")
    mc_d = nc.inline_tensor(mc_np, "mask_c")
    idb_d = nc.inline_tensor(np.eye(128, dtype=BF), "ident_bf")

    with tile.TileContext(nc) as tc:
        with tc.tile_pool(name="wpool", bufs=1) as wp, \
             tc.tile_pool(name="pers", bufs=1) as pers, \
             tc.tile_pool(name="ppool", bufs=1, space="PSUM") as pp, \
             tc.tile_pool(name="work", bufs=1) as wk_pool:

            xt = pers.tile([128, NCH, NKT, 512], bf)
            tcos = pers.tile([128, S], bf)
            tsin = pers.tile([128, S], bf)
            wo = pers.tile([128, 2, D], bf)

            qt = pers.tile([128, 2, S], bf)      # RoPE'd scaled q, hd-major
            kt = pers.tile([128, S], bf)         # RoPE'd k, hd-major
            vsm = pers.tile([128, NQT, 128], bf)  # v, S-major [s, hd]
            att = pers.tile([128, 2, S], bf)     # attention out^T, hd-major

            # ---- DMA order: proj weights + chunk-0 first, wo before qi0's
            # out-proj, later chunks streamed. ----
            wq = wp.tile([128, NKT, 2, 128], bf)
            nc.sync.dma_start(out=wq[:, 0:4], in_=wq_d[:, 0:4])
            sl0 = slice(0, 512)
            nc.sync.dma_start(out=tcos[:, sl0], in_=tcos_d[:, sl0])
            nc.sync.dma_start(out=tsin[:, sl0], in_=tsin_d[:, sl0])
            nc.sync.dma_start(out=xt[:, 0, 0:4], in_=xt_d[:, 0, 0:4])
            for g in range(1, 4):
                nc.sync.dma_start(out=wq[:, 4 * g:4 * g + 4],
                                  in_=wq_d[:, 4 * g:4 * g + 4])
                nc.sync.dma_start(out=xt[:, 0, 4 * g:4 * g + 4],
                                  in_=xt_d[:, 0, 4 * g:4 * g + 4])
            wvt = wp.tile([128, NKT, 128], bf)
            nc.sync.dma_start(out=wvt[:], in_=wv_d[:])
            wkt = wp.tile([128, NKT, 128], bf)
            nc.sync.dma_start(out=wkt[:], in_=wk_d[:])
            mli = wp.tile([128, 128], bf)
            nc.sync.dma_start(out=mli[:], in_=mli_d[:])
            mc = wp.tile([128, 128], bf)
            nc.sync.dma_start(out=mc[:], in_=mc_d[:])
            idb = wp.tile([128, 128], bf)
            nc.sync.dma_start(out=idb[:], in_=idb_d[:])
            lamn = wp.tile([1, 2], f32)
            nc.sync.dma_start(out=lamn[:], in_=lamn_d[:])
            lamb = wp.tile([128, 2], f32)
            nc.gpsimd.partition_broadcast(lamb[:], lamn[:])
            nc.sync.dma_start(out=wo[:], in_=wo_d[:])

            def dma_chunk(ch):
                sl = slice(ch * 512, (ch + 1) * 512)
                nc.sync.dma_start(out=xt[:, ch], in_=xt_d[:, ch])
                nc.sync.dma_start(out=tcos[:, sl], in_=tcos_d[:, sl])
                nc.sync.dma_start(out=tsin[:, sl], in_=tsin_d[:, sl])

            def rope(ps, outt, sl, mul_eng, comb_eng):
                fb = wk_pool.tile([128, 512], bf, tag="fb", bufs=3)
                nc.scalar.copy(out=fb[:], in_=ps[:])
                m1 = wk_pool.tile([128, 512], bf, tag="m1", bufs=2)
                m2 = wk_pool.tile([128, 512], bf, tag="m2", bufs=2)
                # m2[p] = f[swap(p)] * sin_signed[swap(p)]  (sign in the table,
                # table read partition-aligned with the swapped operand)
                mul_eng.tensor_mul(m2[0:64, :], fb[64:128, :], tsin[64:128, sl])
                mul_eng.tensor_mul(m2[64:128, :], fb[0:64, :], tsin[0:64, sl])
                nc.gpsimd.tensor_mul(m1[:], fb[:], tcos[:, sl])
                comb_eng.tensor_add(outt[:], m1[:], m2[:])

            def proj_chunk(ch):
                sl = slice(ch * 512, (ch + 1) * 512)
                rhs = [xt[:, ch, kti, :] for kti in range(NKT)]
                ps_q0 = pp.tile([128, 512], f32, tag="proj", bufs=2)
                for kti in range(NKT):
                    nc.tensor.matmul(ps_q0[:], wq[:, kti, 0, :], rhs[kti],
                                     start=(kti == 0), stop=(kti == NKT - 1))
                ps_v = pp.tile([128, 512], f32, tag="proj", bufs=2)
                for kti in range(NKT):
                    nc.tensor.matmul(ps_v[:], wvt[:, kti, :], rhs[kti],
                                     start=(kti == 0), stop=(kti == NKT - 1))
                rope(ps_q0, qt[:, 0, sl], sl, nc.vector, nc.vector)
                vtmp = wk_pool.tile([128, 512], bf, tag="vtmp", bufs=2)
                nc.scalar.copy(out=vtmp[:], in_=ps_v[:])

                ps_k = pp.tile([128, 512], f32, tag="proj", bufs=2)
                for kti in range(NKT):
                    nc.tensor.matmul(ps_k[:], wkt[:, kti, :], rhs[kti],
                                     start=(kti == 0), stop=(kti == NKT - 1))
                ps_q1 = pp.tile([128, 512], f32, tag="proj", bufs=2)
                for kti in range(NKT):
                    nc.tensor.matmul(ps_q1[:], wq[:, kti, 1, :], rhs[kti],
                                     start=(kti == 0), stop=(kti == NKT - 1))
                rope(ps_k, kt[:, sl], sl, nc.vector, nc.vector)
                rope(ps_q1, qt[:, 1, sl], sl, nc.vector, nc.vector)

                # v transpose to S-major via PE
                ps_tv = pp.tile([128, 4, 128], bf, tag="av", bufs=2)
                for j in range(4):
                    nc.tensor.transpose(ps_tv[:, j, :],
                                        vtmp[:, 128 * j:128 * (j + 1)], idb[:])
                nc.vector.tensor_copy(out=vsm[:, 4 * ch:4 * (ch + 1), :],
                                      in_=ps_tv[:])

            def attention_scores(qi):
                qsl = slice(qi * 128, (qi + 1) * 128)
                kw = min(qi + 1, 5)
                w = kw * 128

                es = []
                zs = []
                k0 = max(0, qi - 4) * 128
                for h in range(2):
                    for half in range(2):
                        hp = slice(64 * half, 64 * half + 64)
                        lhs = qt[hp, h, qsl]
                        ps = pp.tile([128, 640], f32, tag="s", bufs=2)
                        if qi >= 4:
                            # bank A: tile0 masked (window edge), tiles 1-3 free
                            nc.tensor.matmul(ps[:, 0:128], idb[:], mli[:],
                                             start=True, stop=False)
                            nc.tensor.matmul(ps[:, 0:128], lhs,
                                             kt[hp, k0:k0 + 128],
                                             start=False, stop=True)
                            nc.tensor.matmul(ps[:, 128:512], lhs,
                                             kt[hp, k0 + 128:k0 + 512],
                                             start=True, stop=True)
                            # bank B: tile4 causal
                            nc.tensor.matmul(ps[:, 512:640], idb[:], mc[:],
                                             start=True, stop=False)
                            nc.tensor.matmul(ps[:, 512:640], lhs,
                                             kt[hp, k0 + 512:k0 + 640],
                                             start=False, stop=True)
                        else:
                            if qi > 0:
                                nc.tensor.matmul(ps[:, 0:qi * 128], lhs,
                                                 kt[hp, 0:qi * 128],
                                                 start=True, stop=True)
                            nc.tensor.matmul(ps[:, qi * 128:w], idb[:], mc[:],
                                             start=True, stop=False)
                            nc.tensor.matmul(ps[:, qi * 128:w], lhs,
                                             kt[hp, qi * 128:w],
                                             start=False, stop=True)
                        e = wk_pool.tile([128, 640], bf, tag="e", bufs=6)
                        z = wk_pool.tile([128, 1], f32, tag="z", bufs=8)
                        nc.scalar.activation(out=e[:, 0:w], in_=ps[:, 0:w],
                                             func=EXP, accum_out=z[:])
                        es.append(e)
                        zs.append(z)

                gns = []
                for h in range(2):
                    e1, e2 = es[2 * h], es[2 * h + 1]
                    z1, z2 = zs[2 * h], zs[2 * h + 1]
                    r2 = wk_pool.tile([128, 1], f32, tag="r2", bufs=4)
                    nc.vector.reciprocal(out=r2[:], in_=z2[:])
                    ctmp = wk_pool.tile([128, 1], f32, tag="ctmp", bufs=4)
                    nc.gpsimd.tensor_mul(ctmp[:], z1[:], lamb[:, h:h + 1])
                    cneg = wk_pool.tile([128, 1], f32, tag="cneg", bufs=4)
                    nc.gpsimd.tensor_mul(cneg[:], ctmp[:], r2[:])
                    g0 = wk_pool.tile([128, 640], bf, tag="g0", bufs=2)
                    nc.vector.scalar_tensor_tensor(
                        out=g0[:, 0:w], in0=e2[:, 0:w], scalar=cneg[:],
                        in1=e1[:, 0:w], op0=MULT, op1=ADD)
                    g = wk_pool.tile([128, 640], bf, tag="g", bufs=2)
                    dsum = wk_pool.tile([128, 1], f32, tag="dsum", bufs=4)
                    nc.vector.tensor_scalar(
                        out=g[:, 0:w], in0=g0[:, 0:w], scalar1=0.0,
                        scalar2=0.0, op0=MAX, op1=ADD, accum_out=dsum[:])
                    dtmp = wk_pool.tile([128, 1], f32, tag="dtmp", bufs=4)
                    nc.vector.scalar_tensor_tensor(
                        out=dtmp[:], in0=z1[:], scalar=1e-6, in1=dsum[:],
                        op0=MULT, op1=ADD)
                    recd = wk_pool.tile([128, 1], f32, tag="recd", bufs=4)
                    nc.vector.reciprocal(out=recd[:], in_=dtmp[:])
                    gn = wk_pool.tile([128, 640], bf, tag="gn", bufs=2)
                    nc.vector.tensor_scalar(
                        out=gn[:, 0:w], in0=g[:, 0:w], scalar1=recd[:],
                        scalar2=0.0, op0=MULT, op1=ADD)
                    gns.append(gn)
                return gns

            def attention_av(qi, gns):
                qsl = slice(qi * 128, (qi + 1) * 128)
                kw = min(qi + 1, 5)
                w = kw * 128
                kstart = max(0, qi - 4)

                ps_av = pp.tile([128, 2, 128], f32, tag="av", bufs=2)
                for h in range(2):
                    gn = gns[h]
                    ps_tr = pp.tile([128, 640], bf, tag="s", bufs=2)
                    for j in range(kw):
                        nc.tensor.transpose(ps_tr[:, 128 * j:128 * (j + 1)],
                                            gn[:, 128 * j:128 * (j + 1)], idb[:])
                    gts = wk_pool.tile([128, 640], bf, tag="gts", bufs=2)
                    nc.vector.tensor_copy(out=gts[:, 0:w], in_=ps_tr[:, 0:w])
                    for j in range(kw):
                        nc.tensor.matmul(ps_av[:, h, :], vsm[:, kstart + j, :],
                                         gts[:, 128 * j:128 * (j + 1)],
                                         start=(j == 0), stop=(j == kw - 1))
                nc.vector.tensor_copy(out=att[:, :, qsl], in_=ps_av[:])

            def out_proj(qi):
                qsl = slice(qi * 128, (qi + 1) * 128)
                so = wk_pool.tile([128, D], f16, tag="so", bufs=2)
                for dch in range(4):
                    dsl = slice(dch * 512, (dch + 1) * 512)
                    ps_o = pp.tile([128, 512], f32, tag="s", bufs=2)
                    nc.tensor.matmul(ps_o[:], att[:, 0, qsl], wo[:, 0, dsl],
                                     start=True, stop=False)
                    nc.tensor.matmul(ps_o[:], att[:, 1, qsl], wo[:, 1, dsl],
                                     start=False, stop=True)
                    if dch % 2 == 0:
                        nc.scalar.copy(out=so[:, dsl], in_=ps_o[:])
                    else:
                        nc.vector.tensor_copy(out=so[:, dsl], in_=ps_o[:])
                nc.sync.dma_start(out=out_d[qsl, :], in_=so[:])

            # ---- fused streaming schedule: out-proj(qi-1)'s matmuls are
            # emitted between scores(qi) and av(qi) so the in-order PE queue
            # has work while the softmax chain for qi runs on ACT/DVE/Pool.
            for ch in range(NCH):
                if ch + 1 < NCH:
                    dma_chunk(ch + 1)
                proj_chunk(ch)
                for qi in range(4 * ch, 4 * ch + 4):
                    gns = attention_scores(qi)
                    if qi > 0:
                        out_proj(qi - 1)
                    attention_av(qi, gns)
            out_proj(NQT - 1)

    nc.compile()
    return nc


def get_program():
    if "nc" not in _CACHE:
        _CACHE["nc"] = _build_program()
    return _CACHE["nc"]


def _prep_inputs(x, Wq, Wk, Wv, Wo, lam):
    # [128, NCH, NKT, 512]: chunk-major so each chunk's DMA is one dense block
    xt = np.ascontiguousarray(x.reshape(S, D).T.astype(BF)
                              .reshape(NKT, 128, NCH, 512).transpose(1, 2, 0, 3))
    in_maps = []
    for c in range(N_CORES):
        h0 = 2 * c
        kv = c // 2
        wq_c = np.ascontiguousarray(
            (Wq[:, h0 * 128:(h0 + 2) * 128] * 0.125).astype(BF)
            .reshape(NKT, 128, 2, 128).transpose(1, 0, 2, 3))
        wk_c = np.ascontiguousarray(
            Wk[:, kv * 128:(kv + 1) * 128].astype(BF)
            .reshape(NKT, 128, 128).transpose(1, 0, 2))
        wv_c = np.ascontiguousarray(
            Wv[:, kv * 128:(kv + 1) * 128].astype(BF)
            .reshape(NKT, 128, 128).transpose(1, 0, 2))
        wo_c = np.ascontiguousarray(
            Wo[h0 * 128:(h0 + 2) * 128, :].astype(BF)
            .reshape(2, 128, D).transpose(1, 0, 2))
        lamn_c = np.array([[-float(lam[h0]), -float(lam[h0 + 1])]], dtype=np.float32)
        in_maps.append({"xt": xt, "wq": wq_c, "wk": wk_c, "wv": wv_c,
                        "wo": wo_c, "lamn": lamn_c})
    return in_maps


def kernel(x, Wq, Wk, Wv, Wo, lam):
    from concourse.bass_utils import run_bass_kernel_spmd

    nc = get_program()
    in_maps = _prep_inputs(np.asarray(x), np.asarray(Wq), np.asarray(Wk),
                           np.asarray(Wv), np.asarray(Wo), np.asarray(lam))
    res = run_bass_kernel_spmd(nc, in_maps, list(range(N_CORES)))
    out = np.zeros((S, D), dtype=np.float32)
    for c in range(N_CORES):
        out += res.results[c]["outp"].astype(np.float32)
    return out.reshape(1, S, D)
